# revision 20
# baseline (speedup 1.0000x reference)
"""BiMamba (bidirectional Mamba block + LN + FFN) Trainium2 Bass kernel.

Sharding (8 cores): 4 scan-sequences (fwd/bwd x batch, bwd fed host-flipped x)
x 2 halves of d_inner. Feature-on-partitions / time-on-free throughout.

v3 changes vs v2:
 - NO gpsimd elementwise ops: gpsimd shares an exclusive SBUF port pair with
   the DVE; any gpsimd tensor op fully blocks concurrent DVE 2-src ops
   (measured 3.2x slowdown). gpsimd only triggers collectives now.
 - scan-phase elementwise quad-batched: one [P, 4096] op covers the 4
   d-blocks of a half (b-mult / scan / prod-mult); independent streams are
   separated in the scan by forcing a=0 at each stream's first column.
   B/C row-broadcast tiles are read 4x via a zero-stride AP view.
 - y += D*xc folded into the scan PSUM accumulation with diag(D) matmuls
   (host passes Ddiag_t); yg merge is a single quad TT per half.
 - back-end collectives split in row-halves (AR) and the ReduceScatter in
   two pieces so collective latency overlaps compute; final output rows
   remapped accordingly on the host.
 - out_proj/FFN weight preloads: ow trickles during the scan, w1/w2 are
   DMA'd at the start of the back end (hidden under out_proj + AllReduce).
"""
import sys, os, types, contextlib, ctypes

sys.path.insert(0, "/opt/trn_rl_repo")
import numpy as np
import ml_dtypes

BF16 = ml_dtypes.bfloat16

D_MODEL = 1024
D_STATE = 16
D_CONV = 4
D_INNER = 2048
DT_RANK = 64
L = 1024
HALF = D_INNER // 2          # 1024 d_inner per core
P = 128
NJ = HALF // P               # 8 d-blocks per core half
TCH = 512                    # matmul t-chunk
NT = L // TCH
KD = D_MODEL // P            # 8 k-chunks over d_model
FF_SLICE = 1024              # ffn hidden slice per core
QL = 4 * L                   # quad free width (4 d-blocks)

_QUADS = [[0, 1, 4, 5], [2, 3, 6, 7]]
_PAIRS = [[0, 1], [2, 3], [4, 5], [6, 7]]

# column segments for the two reversal-symmetric back-end chunks
_HSEGS = [((0, 256), (768, 1024)), ((256, 512), (512, 768))]


def _install_ntff_hook_shim(so_path="/opt/axon/libaxon_pjrt.so"):
    if "antenv.axon_hooks" in sys.modules:
        return
    try:
        lib = ctypes.CDLL(so_path)
    except OSError:
        return
    if not hasattr(lib, "axon_start_nrt_profile"):
        return
    lib.axon_start_nrt_profile.argtypes = [ctypes.POINTER(ctypes.c_int64), ctypes.c_size_t]
    lib.axon_start_nrt_profile.restype = ctypes.c_int64
    lib.axon_stop_nrt_profile.argtypes = [ctypes.c_char_p]
    lib.axon_stop_nrt_profile.restype = ctypes.c_int64

    @contextlib.contextmanager
    def _hook(output_dir, device_ids):
        import jax
        jax.devices()
        if device_ids:
            ids = (ctypes.c_int64 * len(device_ids))(*device_ids)
            rc = lib.axon_start_nrt_profile(ids, len(device_ids))
        else:
            rc = lib.axon_start_nrt_profile(None, 0)
        if rc != 0:
            raise RuntimeError(f"axon_start_nrt_profile rc={rc}")
        try:
            yield
        finally:
            n = lib.axon_stop_nrt_profile(str(output_dir).encode())
            print(f"profile: {n} file(s) written to {output_dir}", file=sys.stderr)

    mod = types.ModuleType("antenv.axon_hooks")
    mod.get_axon_ntff_profile_hook = lambda: _hook
    mod.set_axon_ntff_profile_hook = lambda h: None
    sys.modules["antenv.axon_hooks"] = mod


def _build_nc():
    from concourse import bacc, tile, mybir

    f32 = mybir.dt.float32
    bf16 = mybir.dt.bfloat16
    u8 = mybir.dt.uint8
    Alu = mybir.AluOpType
    Act = mybir.ActivationFunctionType

    nc = bacc.Bacc("TRN2", target_bir_lowering=False, debug=False, num_devices=8)

    def din(name, shape, dt=None):
        return nc.dram_tensor(name, list(shape), dt or f32, kind="ExternalInput").ap()

    xT = din("xT", (D_MODEL, L), bf16)
    w_in_h = din("w_in_h", (2 * NJ, P, D_MODEL), bf16)   # [fb][kp, k*128+mp]
    convw_cols = din("convw_cols", (P, NJ * D_CONV))
    convb_cols = din("convb_cols", (P, NJ))
    xpw_t = din("xpw_t", (NJ, P, P), bf16)               # cols: dt64|B16|pad|C16|pad
    dtw_t = din("dtw_t", (NJ, DT_RANK, P), bf16)
    dtb_cols = din("dtb_cols", (P, NJ))
    A_cols = din("A_cols", (P, NJ * D_STATE))
    Ddiag_t = din("Ddiag_t", (NJ, P, P), bf16)           # diag(D) per d-block
    ow_h = din("ow_h", (NJ, P, D_MODEL), bf16)           # [k][kp, dm]
    lng_cols = din("lng_cols", (P, KD))
    lnb_cols = din("lnb_cols", (P, KD))
    w1_h = din("w1_h", (KD, P, 4 * D_MODEL), bf16)       # full [k][kp, h]
    b1_cols = din("b1_cols", (P, 4 * KD))
    w2_h = din("w2_h", (KD, P, 4 * D_MODEL), bf16)       # [g][kp, (q, dm)]
    b2_cols = din("b2_cols", (P, KD))
    ident_b = din("ident_b", (P, P), bf16)
    dirmask = din("dirmask", (P, TCH), u8)               # 1 on bwd cores
    onescol = din("onescol", (P, 1), bf16)               # 2^-10 (1/1024)
    onesrow = din("onesrow", (1, P), bf16)               # 1.0

    out_m = nc.dram_tensor("out_m", [D_MODEL, 2 * TCH // 4], f32,
                           kind="ExternalOutput").ap()

    with tile.TileContext(nc) as tc:
        with contextlib.ExitStack() as stk:
            cpool = stk.enter_context(tc.tile_pool(name="cpool", bufs=1))
            dram = stk.enter_context(tc.tile_pool(name="dram", bufs=1, space="DRAM"))

            def cload(src, shape, tag, dt=f32):
                t = cpool.tile(list(shape), dt, tag=tag, name=tag)
                nc.sync.dma_start(t[:], src)
                return t

            A_sb = cload(A_cols[:], (P, NJ * D_STATE), "A_sb")
            dtb_sb = cload(dtb_cols[:], (P, NJ), "dtb_sb")
            convb_sb = cload(convb_cols[:], (P, NJ), "convb_sb")
            convw_sb = cload(convw_cols[:], (P, NJ * D_CONV), "convw_sb")
            lng_sb = cload(lng_cols[:], (P, KD), "lng_sb")
            lnb_sb = cload(lnb_cols[:], (P, KD), "lnb_sb")
            b1_sb = cload(b1_cols[:], (P, 4 * KD), "b1_sb")
            b2_sb = cload(b2_cols[:], (P, KD), "b2_sb")
            ident_sb = cload(ident_b[:], (P, P), "ident_sb", bf16)
            dirmask_sb = cload(dirmask[:], (P, TCH), "dirmask_sb", u8)
            onescol_sb = cload(onescol[:], (P, 1), "onescol_sb", bf16)
            onesrow_sb = cload(onesrow[:], (1, P), "onesrow_sb", bf16)
            Ddiag_sb = [cload(Ddiag_t[j], (P, P), f"Ddiag{j}", bf16)
                        for j in range(NJ)]

            warm_in = dram.tile([1, 256], bf16, name="warm_in")
            warm_out = dram.tile([1, 256], bf16, name="warm_out")
            dbl_in_d = dram.tile([P, L], bf16, name="dbl_in_d")
            dbl_out_d = dram.tile([P, L], bf16, name="dbl_out_d")
            bcB_d = dram.tile([D_STATE, L], bf16, name="bcB_d")
            bcC_d = dram.tile([D_STATE, L], bf16, name="bcC_d")
            arm_in = [dram.tile([4, D_MODEL, TCH // 4], bf16, name=f"arm_in{h}")
                      for h in range(2)]
            rs_out = [dram.tile([D_MODEL, TCH // 4], bf16, name=f"rs_out{h}")
                      for h in range(2)]

            def mm_accum(ps, lw_list, rhs_of_k, n_k):
                for k in range(n_k):
                    nc.tensor.matmul(ps[:], lw_list[k][:], rhs_of_k(k),
                                     start=(k == 0), stop=(k == n_k - 1))

            # yg survives into the back end; sz/wv/delta live only front+scan
            per_pool = stk.enter_context(tc.tile_pool(name="per_pool", bufs=1))
            yg_q = [per_pool.tile([P, QL], bf16, tag=f"ygq{h}", name=f"ygq{h}")
                    for h in range(2)]

            # out_proj/FFN weight pool (tiles created lazily: ow during scan,
            # w1/w2 at back-end start)
            wq = stk.enter_context(tc.tile_pool(name="wq", bufs=1))

            def _wtiles(src, n_m, n_k, pfx, pool=None):
                pool = pool or wq
                rows, pend = [], []
                for m in range(n_m):
                    row = []
                    for k in range(n_k):
                        t = pool.tile([P, P], bf16, tag=f"{pfx}{m}_{k}",
                                      name=f"{pfx}{m}_{k}")
                        pend.append((t, src[k, m]))
                        row.append(t)
                    rows.append(row)
                return rows, pend

            # ---------------- P1..P4: produce xc, sz, delta, wv ----------------
            with tc.tile_pool(name="xc_pool", bufs=1) as xc_pool, \
                 tc.tile_pool(name="xt_pool", bufs=1) as xt_pool:
                xc_q = [xc_pool.tile([P, QL], bf16, tag=f"xcq{h}", name=f"xcq{h}")
                        for h in range(2)]
                sz_q = [xc_pool.tile([P, QL], bf16, tag=f"szq{h}", name=f"szq{h}")
                        for h in range(2)]
                wv_q = [xc_pool.tile([P, QL], bf16, tag=f"wvq{h}", name=f"wvq{h}")
                        for h in range(2)]
                delta_q = [xc_pool.tile([P, QL], bf16, tag=f"dq{h}",
                                        name=f"dq{h}") for h in range(2)]
                with tc.tile_pool(name="p1t", bufs=1) as p1t, \
                     tc.tile_pool(name="psumA", bufs=4, space="PSUM") as psumA:
                    xts = []
                    for k in range(KD):
                        xt_k = xt_pool.tile([P, L], bf16, tag=f"xt{k}",
                                            name=f"xt{k}")
                        nc.sync.dma_start(xt_k[:], xT[k * P:(k + 1) * P, :])
                        xts.append(xt_k)

                    def in_proj_block(fb):
                        lw = p1t.tile([P, D_MODEL], bf16, tag="lwb",
                                      name=f"lwb{fb}", bufs=2)
                        nc.sync.dma_start(lw[:], w_in_h[fb])
                        pss = []
                        for t in range(NT):
                            ps = psumA.tile([P, TCH], f32, tag="ps",
                                            name=f"inp{fb}_{t}")
                            for k in range(KD):
                                nc.tensor.matmul(
                                    ps[:], lw[:, k * P:(k + 1) * P],
                                    xts[k][:, t * TCH:(t + 1) * TCH],
                                    start=(k == 0), stop=(k == KD - 1))
                            pss.append(ps)
                        return pss

                    # P1: xc half + conv + silu -> xc_q slices
                    for j in range(NJ):
                        hb, s = j // 4, j % 4
                        xcp = p1t.tile([P, L + D_CONV - 1], bf16, tag="xcp",
                                       name=f"xcp{j}", bufs=2)
                        nc.vector.memset(xcp[:, 0:D_CONV - 1], 0.0)
                        for t, ps in enumerate(in_proj_block(j)):
                            nc.scalar.copy(
                                xcp[:, D_CONV - 1 + t * TCH:
                                    D_CONV - 1 + (t + 1) * TCH],
                                ps[:])
                        # aligned copy of the odd-shifted window: keeps every
                        # conv tap on a 4B-aligned stream (2x DVE mode)
                        xcp1 = p1t.tile([P, L + 2], bf16, tag="xcp1",
                                        name=f"xcp1_{j}", bufs=2)
                        nc.scalar.copy(xcp1[:], xcp[:, 1:L + 3])
                        cacc = p1t.tile([P, L], bf16, tag="cacc", name=f"cacc{j}",
                                        bufs=2)
                        nc.vector.tensor_scalar_mul(
                            cacc[:], xcp[:, 0:L],
                            convw_sb[:, j * D_CONV:j * D_CONV + 1])
                        for i, (tile, o) in enumerate(
                                ((xcp1, 0), (xcp, 2), (xcp1, 2)), start=1):
                            nc.vector.scalar_tensor_tensor(
                                cacc[:], tile[:, o:o + L],
                                convw_sb[:, j * D_CONV + i:j * D_CONV + i + 1],
                                cacc[:], Alu.mult, Alu.add)
                        nc.scalar.activation(xc_q[hb][:, s * L:(s + 1) * L],
                                             cacc[:], Act.Silu,
                                             bias=convb_sb[:, j:j + 1])

                    # P2: x_proj partial over own xc half -> pairwise AllReduce
                    dblp = p1t.tile([P, L], bf16, tag="dblp", name="dblp", bufs=1)
                    for t in range(NT):
                        ps = psumA.tile([P, TCH], f32, tag="ps", name=f"xproj{t}")
                        for k in range(NJ):
                            lw = p1t.tile([P, P], bf16, tag="xpw",
                                          name=f"xpw{t}_{k}", bufs=2)
                            nc.sync.dma_start(lw[:], xpw_t[k])
                            nc.tensor.matmul(
                                ps[:], lw[:],
                                xc_q[k // 4][:, (k % 4) * L + t * TCH:
                                             (k % 4) * L + (t + 1) * TCH],
                                start=(k == 0), stop=(k == NJ - 1))
                        nc.scalar.copy(dblp[:, t * TCH:(t + 1) * TCH], ps[:])
                    nc.sync.dma_start(dbl_in_d[:], dblp[:])
                    nc.gpsimd.collective_compute(
                        "AllReduce", Alu.add, replica_groups=_PAIRS,
                        ins=[dbl_in_d[:]], outs=[dbl_out_d[:]])

                    # P4: dt_proj + softplus -> delta_q; wv_q; B/C rows.
                    # Only j0/j1 (first scan sub-half) here; the rest is
                    # deferred into the wave loop (engines have slack there).
                    dbl_sb = xc_pool.tile([P, L], bf16, tag="dbl_sb",
                                          name="dbl_sb", bufs=1)
                    nc.sync.dma_start(dbl_sb[:], dbl_out_d[:])
                    bcB_sb = p1t.tile([D_STATE, L], bf16, tag="bcB_sb",
                                      name="bcB_sb", bufs=1)
                    nc.scalar.copy(bcB_sb[:], dbl_sb[DT_RANK:DT_RANK + D_STATE, :])
                    nc.sync.dma_start(bcB_d[:], bcB_sb[:])
                    bcC_sb = p1t.tile([D_STATE, L], bf16, tag="bcC_sb",
                                      name="bcC_sb", bufs=1)
                    nc.scalar.copy(bcC_sb[:], dbl_sb[96:96 + D_STATE, :])
                    nc.sync.dma_start(bcC_d[:], bcC_sb[:])
                    for j in range(2):
                        hb, s = j // 4, j % 4
                        lw = p1t.tile([DT_RANK, P], bf16, tag="dtw",
                                      name=f"dtw{j}", bufs=2)
                        nc.sync.dma_start(lw[:], dtw_t[j])
                        spt = p1t.tile([P, L], bf16, tag="spt",
                                       name=f"spt{j}", bufs=2)
                        for t in range(NT):
                            ps = psumA.tile([P, TCH], f32, tag="ps",
                                            name=f"dtp{j}_{t}")
                            nc.tensor.matmul(
                                ps[:], lw[:],
                                dbl_sb[0:DT_RANK, t * TCH:(t + 1) * TCH],
                                start=True, stop=True)
                            nc.scalar.activation(spt[:, t * TCH:(t + 1) * TCH],
                                                 ps[:], Act.Exp,
                                                 bias=dtb_sb[:, j:j + 1])
                        nc.scalar.activation(
                            delta_q[hb][:, s * L:(s + 1) * L],
                            spt[:], Act.Ln, bias=1.0)
                    nc.vector.tensor_tensor(wv_q[0][:, 0:2 * L],
                                            delta_q[0][:, 0:2 * L],
                                            xc_q[0][:, 0:2 * L], Alu.mult)

                ow_big = [wq.tile([P, D_MODEL], bf16, tag=f"owb{k}",
                                  name=f"owb{k}") for k in range(NJ)]
                ow_pending = [(ow_big[k], ow_h[k]) for k in range(NJ)]

                # ------------- P5: scan (pair-width waves, deferred z/dt) ----
                with tc.tile_pool(name="pscan", bufs=1, space="PSUM") as pscan, \
                     tc.tile_pool(name="zps", bufs=2, space="PSUM") as zps, \
                     tc.tile_pool(name="tpool", bufs=1) as tpool:

                    def dtsp_thunk(j):
                        def f():
                            hb, s = j // 4, j % 4
                            lw = tpool.tile([DT_RANK, P], bf16, tag="dtwl",
                                            name=f"dtwl{j}", bufs=2)
                            nc.sync.dma_start(lw[:], dtw_t[j])
                            spt = tpool.tile([P, L], bf16, tag="sptl",
                                             name=f"sptl{j}", bufs=2)
                            for t in range(NT):
                                ps = zps.tile([P, TCH], f32, tag="zp",
                                              name=f"dtpl{j}_{t}")
                                nc.tensor.matmul(
                                    ps[:], lw[:],
                                    dbl_sb[0:DT_RANK, t * TCH:(t + 1) * TCH],
                                    start=True, stop=True)
                                nc.scalar.activation(
                                    spt[:, t * TCH:(t + 1) * TCH], ps[:],
                                    Act.Exp, bias=dtb_sb[:, j:j + 1])
                            nc.scalar.activation(
                                delta_q[hb][:, s * L:(s + 1) * L], spt[:],
                                Act.Ln, bias=1.0)
                        return f

                    def wv_thunk(hb2):
                        def f():
                            hb, hf = hb2 // 2, hb2 % 2
                            sl = slice(hf * 2 * L, (hf + 1) * 2 * L)
                            nc.vector.tensor_tensor(wv_q[hb][:, sl],
                                                    delta_q[hb][:, sl],
                                                    xc_q[hb][:, sl], Alu.mult)
                        return f

                    def z_thunk(j):
                        def f():
                            hb, s = j // 4, j % 4
                            lw = tpool.tile([P, D_MODEL], bf16, tag="zlwb",
                                            name=f"zlwb{j}", bufs=2)
                            nc.sync.dma_start(lw[:], w_in_h[NJ + j])
                            for t in range(NT):
                                ps = zps.tile([P, TCH], f32, tag="zp",
                                              name=f"zin{j}_{t}")
                                for k in range(KD):
                                    nc.tensor.matmul(
                                        ps[:], lw[:, k * P:(k + 1) * P],
                                        xts[k][:, t * TCH:(t + 1) * TCH],
                                        start=(k == 0), stop=(k == KD - 1))
                                nc.scalar.activation(
                                    sz_q[hb][:, s * L + t * TCH:
                                              s * L + (t + 1) * TCH],
                                    ps[:], Act.Silu)
                        return f

                    thunks = [z_thunk(0), z_thunk(1), dtsp_thunk(2),
                              dtsp_thunk(3), wv_thunk(1), z_thunk(2),
                              z_thunk(3), dtsp_thunk(4), dtsp_thunk(5),
                              dtsp_thunk(6), dtsp_thunk(7), wv_thunk(2),
                              z_thunk(4), z_thunk(5), wv_thunk(3),
                              z_thunk(6), z_thunk(7)]

                    # warm up the quad ring so the first real RS isn't cold
                    nc.gpsimd.collective_compute(
                        "AllReduce", Alu.add, replica_groups=_QUADS,
                        ins=[warm_in[:]], outs=[warm_out[:]])
                    for hb2 in range(4):
                        hb, hf = hb2 // 2, hb2 % 2
                        off = hf * 2 * L
                        yps2 = pscan.tile([P, 2 * L], f32, tag="yps2",
                                          name=f"yps2_{hb2}", bufs=1)
                        for s2 in range(2):
                            j = hb * 4 + hf * 2 + s2
                            for c2 in range(2):
                                psl = slice((s2 * 2 + c2) * TCH,
                                            (s2 * 2 + c2 + 1) * TCH)
                                nc.tensor.matmul(
                                    yps2[:, psl], Ddiag_sb[j][:],
                                    xc_q[hb][:, off + s2 * L + c2 * TCH:
                                              off + s2 * L + (c2 + 1) * TCH],
                                    start=True, stop=False)
                        for n in range(D_STATE):
                            Bbc = tpool.tile([P, L], bf16, tag="Bbc",
                                             name=f"Bbc{hb2}_{n}", bufs=2)
                            nc.sync.dma_start(
                                Bbc[:],
                                bcB_d[n:n + 1, :].partition_broadcast(P).squeeze(1))
                            Cbc = tpool.tile([P, L], bf16, tag="Cbc",
                                             name=f"Cbc{hb2}_{n}", bufs=2)
                            nc.sync.dma_start(
                                Cbc[:],
                                bcC_d[n:n + 1, :].partition_broadcast(P).squeeze(1))
                            a_2 = tpool.tile([P, 2 * L], bf16, tag="a_2",
                                             name=f"a{hb2}_{n}", bufs=2)
                            for s2 in range(2):
                                j = hb * 4 + hf * 2 + s2
                                nc.scalar.activation(
                                    a_2[:, s2 * L:(s2 + 1) * L],
                                    delta_q[hb][:, off + s2 * L:
                                                off + (s2 + 1) * L],
                                    Act.Exp,
                                    scale=A_sb[:, j * D_STATE + n:
                                               j * D_STATE + n + 1])
                            b_2 = tpool.tile([P, 2 * L], bf16, tag="b_2",
                                             name=f"b{hb2}_{n}", bufs=1)
                            nc.vector.tensor_tensor(
                                b_2[:], wv_q[hb][:, off:off + 2 * L],
                                Bbc[:].unsqueeze(1).broadcast_to([P, 2, L]),
                                Alu.mult)
                            h_2 = tpool.tile([P, 2 * L], bf16, tag="h_2",
                                             name=f"h{hb2}_{n}", bufs=2)
                            for s2 in range(2):
                                nc.vector.tensor_tensor_scan(
                                    h_2[:, s2 * L:(s2 + 1) * L],
                                    a_2[:, s2 * L:(s2 + 1) * L],
                                    b_2[:, s2 * L:(s2 + 1) * L], 0.0,
                                    Alu.mult, Alu.add)
                            prod = tpool.tile([P, 2 * L], bf16, tag="prod",
                                              name=f"p{hb2}_{n}", bufs=2)
                            nc.vector.tensor_tensor(
                                prod[:], h_2[:],
                                Cbc[:].unsqueeze(1).broadcast_to([P, 2, L]),
                                Alu.mult)
                            for c in range(4):
                                psl = slice(c * TCH, (c + 1) * TCH)
                                nc.tensor.matmul(yps2[:, psl], ident_sb[:],
                                                 prod[:, psl], start=False,
                                                 stop=(n == D_STATE - 1))
                            if n % 2 == 0 and thunks:
                                thunks.pop(0)()
                            for wt, srcap in ow_pending[:1]:
                                nc.sync.dma_start(wt[:], srcap)
                            del ow_pending[:1]
                        yb2 = tpool.tile([P, 2 * L], bf16, tag="yb2",
                                         name=f"yb{hb2}", bufs=1)
                        nc.scalar.copy(yb2[:], yps2[:])
                        nc.vector.tensor_tensor(
                            yg_q[hb][:, off:off + 2 * L], yb2[:],
                            sz_q[hb][:, off:off + 2 * L], Alu.mult)

            # ------- P6..P8: out_proj + column-RS, then local LN + full FFN ----
            # One quad ReduceScatter (cc_dim=Free) per column-half: each core
            # ends up owning 128 t-columns per half (256 total) with FULL dm,
            # then runs LN + the ENTIRE FFN locally. No AllReduce, no second
            # collective round-trip.
            with tc.tile_pool(name="p6t", bufs=1) as p6t, \
                 tc.tile_pool(name="wq2", bufs=1) as wq2, \
                 tc.tile_pool(name="psumB", bufs=2, space="PSUM") as psumB, \
                 tc.tile_pool(name="pstat", bufs=1, space="PSUM") as pstat:

                w1_big = [wq2.tile([P, 4 * D_MODEL], bf16, tag=f"w1b{k}",
                                   name=f"w1b{k}") for k in range(KD)]
                w2_big = [wq2.tile([P, 4 * D_MODEL], bf16, tag=f"w2b{g}",
                                   name=f"w2b{g}") for g in range(KD)]

                def out_proj_half(ha):
                    sel = slice(0, 4, 3) if ha == 0 else slice(1, 3, 1)
                    for m in range(NJ):
                        ps = psumB.tile([P, TCH], f32, tag="ps",
                                        name=f"op{ha}_{m}")
                        for k in range(NJ):
                            rhs = yg_q[k // 4][:, (k % 4) * L:
                                               (k % 4) * L + L].rearrange(
                                "p (a b) -> p a b", a=4)[:, sel, :]
                            nc.tensor.matmul(ps[:],
                                             ow_big[k][:, m * P:(m + 1) * P],
                                             rhs, start=(k == 0),
                                             stop=(k == NJ - 1))
                        msb = p6t.tile([P, TCH], bf16, tag="msb",
                                       name=f"msb{ha}_{m}", bufs=2)
                        nc.scalar.copy(msb[:], ps[:])
                        nc.vector.copy_predicated(msb[:], dirmask_sb[:],
                                                  ps[:, ::-1])
                        for q in range(4):
                            nc.sync.dma_start(
                                arm_in[ha][q, m * P:(m + 1) * P, :],
                                msb[:, q * (TCH // 4):(q + 1) * (TCH // 4)])
                    nc.gpsimd.collective_compute(
                        "ReduceScatter", Alu.add, replica_groups=_QUADS,
                        ins=[arm_in[ha][:]], outs=[rs_out[ha][:]])

                TO = 2 * TCH // 4            # 256 owned t-columns per core

                def ln_ffn():
                    HT = TO // 2
                    mos, xns, ffhs = [], [], []
                    for m in range(KD):
                        mos.append(p6t.tile([P, TO], bf16, tag=f"mo{m}",
                                            name=f"mo{m}", bufs=1))
                        xns.append(p6t.tile([P, TO], bf16, tag=f"xn{m}",
                                            name=f"xn{m}", bufs=1))
                    for m in range(4 * KD):
                        ffhs.append(p6t.tile([P, TO], bf16, tag=f"ffh{m}",
                                             name=f"ffh{m}", bufs=1))
                    mu_ps = pstat.tile([1, TO], f32, tag="mu_ps", name="mu",
                                       bufs=1)
                    e2_ps = pstat.tile([1, TO], f32, tag="e2_ps", name="e2",
                                       bufs=1)
                    eps_sb = p6t.tile([1, 1], f32, tag="eps_sb", name="eps",
                                      bufs=1)
                    nc.vector.memset(eps_sb[:], 1e-5)

                    for ha in range(2):
                        hsl = slice(ha * HT, (ha + 1) * HT)
                        # stats for this half's owned columns
                        for m in range(KD):
                            mo = mos[m]
                            nc.sync.dma_start(mo[:, hsl],
                                              rs_out[ha][m * P:(m + 1) * P, :])
                            sq = p6t.tile([P, HT], bf16, tag="sq",
                                          name=f"sq{ha}_{m}", bufs=2)
                            nc.scalar.activation(sq[:], mo[:, hsl], Act.Square)
                            nc.tensor.matmul(mu_ps[:, hsl], onescol_sb[:],
                                             mo[:, hsl], start=(m == 0),
                                             stop=(m == KD - 1))
                            nc.tensor.matmul(e2_ps[:, hsl], onescol_sb[:],
                                             sq[:], start=(m == 0),
                                             stop=(m == KD - 1))
                        m2 = p6t.tile([1, HT], f32, tag="m2", name=f"m2{ha}",
                                      bufs=2)
                        nc.scalar.activation(m2[:], mu_ps[:, hsl], Act.Square)
                        var_t = p6t.tile([1, HT], f32, tag="var_t",
                                         name=f"var{ha}", bufs=2)
                        nc.vector.tensor_tensor(var_t[:], e2_ps[:, hsl], m2[:],
                                                Alu.subtract)
                        std_t = p6t.tile([1, HT], f32, tag="std_t",
                                         name=f"std{ha}", bufs=2)
                        nc.scalar.activation(std_t[:], var_t[:], Act.Sqrt,
                                             bias=eps_sb[:])
                        rstd_b = p6t.tile([1, HT], bf16, tag="rstd_b",
                                          name=f"rstd{ha}", bufs=2)
                        with nc.allow_low_precision(reason="bf16 rstd bcast"):
                            nc.vector.reciprocal(rstd_b[:], std_t[:])
                        mean_b = p6t.tile([1, HT], bf16, tag="mean_b",
                                          name=f"mean{ha}", bufs=2)
                        nc.scalar.copy(mean_b[:], mu_ps[:, hsl])
                        mean_ps = pstat.tile([P, HT], f32, tag="mean_ps",
                                             name=f"meanbc{ha}", bufs=1)
                        nc.tensor.matmul(mean_ps[:], onesrow_sb[:], mean_b[:],
                                         start=True, stop=True)
                        rstd_ps = pstat.tile([P, HT], f32, tag="rstd_ps",
                                             name=f"rstdbc{ha}", bufs=1)
                        nc.tensor.matmul(rstd_ps[:], onesrow_sb[:], rstd_b[:],
                                         start=True, stop=True)
                        mean_bc = p6t.tile([P, HT], bf16, tag="mean_bc",
                                           name=f"meanbcs{ha}", bufs=2)
                        nc.scalar.copy(mean_bc[:], mean_ps[:])
                        rstd_bc = p6t.tile([P, HT], bf16, tag="rstd_bc",
                                           name=f"rstdbcs{ha}", bufs=2)
                        nc.scalar.copy(rstd_bc[:], rstd_ps[:])

                        for m in range(KD):
                            t1 = p6t.tile([P, HT], bf16, tag="lnt",
                                          name=f"lnt{ha}_{m}", bufs=2)
                            nc.vector.tensor_tensor(t1[:], mos[m][:, hsl],
                                                    mean_bc[:], Alu.subtract)
                            nc.vector.tensor_tensor(t1[:], t1[:], rstd_bc[:],
                                                    Alu.mult)
                            nc.vector.tensor_scalar(xns[m][:, hsl], t1[:],
                                                    lng_sb[:, m:m + 1],
                                                    lnb_sb[:, m:m + 1],
                                                    Alu.mult, Alu.add)
                        # this half's w1 pass (ha0's hides under the 2nd RS)
                        for m in range(4 * KD):
                            ps = psumB.tile([P, TO], f32, tag="psf",
                                            name=f"f1_{ha}_{m}")
                            for k in range(KD):
                                nc.tensor.matmul(
                                    ps[:, 0:HT],
                                    w1_big[k][:, m * P:(m + 1) * P],
                                    xns[k][:, hsl], start=(k == 0),
                                    stop=(k == KD - 1))
                            nc.scalar.activation(ffhs[m][:, hsl], ps[:, 0:HT],
                                                 Act.Gelu,
                                                 bias=b1_sb[:, m:m + 1])
                    for m in range(KD):
                        ps = psumB.tile([P, TO], f32, tag="psf",
                                        name=f"f2_{m}")
                        for k2 in range(4 * KD):
                            nc.tensor.matmul(
                                ps[:],
                                w2_big[k2 // 4][:, (k2 % 4) * D_MODEL +
                                                m * P:(k2 % 4) * D_MODEL +
                                                (m + 1) * P],
                                ffhs[k2][:], start=(k2 == 0),
                                stop=(k2 == 4 * KD - 1))
                        fob = p6t.tile([P, TO], f32, tag="fob", name=f"fob{m}",
                                       bufs=2)
                        nc.vector.tensor_scalar_add(fob[:], ps[:],
                                                    b2_sb[:, m:m + 1])
                        nc.sync.dma_start(out_m[m * P:(m + 1) * P, :], fob[:])

                out_proj_half(0)
                # weights stream during out_proj(1) + the ReduceScatters
                for k in range(KD):
                    nc.sync.dma_start(w1_big[k][:], w1_h[k])
                out_proj_half(1)
                for g in range(KD):
                    nc.sync.dma_start(w2_big[g][:], w2_h[g])
                ln_ffn()

    nc.compile()
    return nc


def _prep_inputs(inputs):
    """Per-core input dicts. Core c: sequence s=c//2 (s>=2 => time-flipped x),
    d_inner half = c%2. The own half of d_inner is permuted FIRST in every
    d_inner-ordered tensor, so the device kernel is identical on all cores."""
    x = np.asarray(inputs["x"], dtype=np.float32)
    in_proj_w = np.asarray(inputs["in_proj_w"], dtype=np.float32)
    conv_w = np.asarray(inputs["conv_w"], dtype=np.float32)
    conv_b = np.asarray(inputs["conv_b"], dtype=np.float32)
    x_proj_w = np.asarray(inputs["x_proj_w"], dtype=np.float32)
    dt_proj_w = np.asarray(inputs["dt_proj_w"], dtype=np.float32)
    dt_proj_b = np.asarray(inputs["dt_proj_b"], dtype=np.float32)
    A = -np.exp(np.asarray(inputs["A_log"], dtype=np.float32))
    Dp = np.asarray(inputs["D"], dtype=np.float32)
    out_proj_w = np.asarray(inputs["out_proj_w"], dtype=np.float32)
    ln_g = np.asarray(inputs["ln_g"], dtype=np.float32)
    ln_b = np.asarray(inputs["ln_b"], dtype=np.float32)
    ff_w1 = np.asarray(inputs["ff_w1"], dtype=np.float32)
    ff_b1 = np.asarray(inputs["ff_b1"], dtype=np.float32)
    ff_w2 = np.asarray(inputs["ff_w2"], dtype=np.float32)
    ff_b2 = np.asarray(inputs["ff_b2"], dtype=np.float32)

    def cols(v):  # (N,) -> (P, N//P) per-partition column layout
        return np.ascontiguousarray(v.reshape(-1, P).T)

    def tile_w(w, KP, MP):  # (K, M) -> (K//KP, M//MP, KP, MP) bf16
        K, M = w.shape
        return np.ascontiguousarray(
            w.reshape(K // KP, KP, M // MP, MP).transpose(0, 2, 1, 3)
        ).astype(BF16)

    in_maps = []
    for c in range(8):
        s, half = c // 2, c % 2
        xb = x[s] if s < 2 else x[s - 2][::-1]
        perm = np.arange(D_INNER).reshape(2, HALF)
        own = np.concatenate([perm[half], perm[1 - half]])[:HALF]

        wxc = in_proj_w[:, own]                               # (1024, 1024)
        wz = in_proj_w[:, D_INNER + own]                      # (1024, 1024)
        w_in = np.concatenate([wxc, wz], axis=1)              # (1024, 2048)
        w_in_h = np.ascontiguousarray(
            w_in.reshape(KD, P, 2 * NJ, P).transpose(2, 1, 0, 3)
            .reshape(2 * NJ, P, D_MODEL)).astype(BF16)        # (16 fb, P, 1024)

        cw = conv_w[own]  # (1024, 4) -> (P, 8*4): col j*4+i = w[jP+p, i]
        convw_cols = np.ascontiguousarray(
            cw.reshape(NJ, P, D_CONV).transpose(1, 0, 2).reshape(P, NJ * D_CONV))

        g = (c & 1) + 2 * (c >> 2)

        Ddiag = np.stack([np.diag(Dp[own][j * P:(j + 1) * P])
                          for j in range(NJ)]).astype(BF16)

        in_maps.append({
            "xT": np.ascontiguousarray(xb.T).astype(BF16),
            "w_in_h": w_in_h,
            "convw_cols": convw_cols,
            "convb_cols": cols(conv_b[own]),
            "xpw_t": np.ascontiguousarray(
                np.concatenate([
                    x_proj_w[own][:, :DT_RANK + D_STATE],
                    np.zeros((HALF, D_STATE), np.float32),
                    x_proj_w[own][:, DT_RANK + D_STATE:],
                    np.zeros((HALF, D_STATE), np.float32),
                ], axis=1).reshape(NJ, P, P)).astype(BF16),
            "dtw_t": np.ascontiguousarray(
                dt_proj_w[:, own].reshape(DT_RANK, NJ, P).transpose(1, 0, 2)
            ).astype(BF16),
            "dtb_cols": cols(dt_proj_b[own]),
            "A_cols": np.ascontiguousarray(
                A[own].reshape(NJ, P, D_STATE).transpose(1, 0, 2).reshape(
                    P, NJ * D_STATE)),
            "Ddiag_t": Ddiag,
            "ow_h": np.ascontiguousarray(
                out_proj_w[own].reshape(NJ, P, D_MODEL)).astype(BF16),
            "lng_cols": cols(ln_g),
            "lnb_cols": cols(ln_b),
            "w1_h": np.ascontiguousarray(
                ff_w1.reshape(KD, P, 4 * D_MODEL)).astype(BF16),
            "b1_cols": cols(ff_b1),
            "w2_h": np.ascontiguousarray(
                ff_w2.reshape(KD, 4, P, D_MODEL).transpose(0, 2, 1, 3)
                .reshape(KD, P, 4 * D_MODEL)).astype(BF16),
            "b2_cols": cols(ff_b2),
            "ident_b": np.eye(P, dtype=np.float32).astype(BF16),
            "dirmask": np.full((P, TCH), 1 if s >= 2 else 0, np.uint8),
            "onescol": np.full((P, 1), 1.0 / 1024.0, np.float32).astype(BF16),
            "onesrow": np.ones((1, P), np.float32).astype(BF16),
        })
    return in_maps


_NC_CACHE = {}


def _get_nc():
    if "nc" not in _NC_CACHE:
        _NC_CACHE["nc"] = _build_nc()
    return _NC_CACHE["nc"]


def run(inputs, trace=False):
    _install_ntff_hook_shim()
    from concourse import bass_utils
    nc = _get_nc()
    in_maps = _prep_inputs(inputs)
    res = bass_utils.run_bass_kernel_spmd(nc, in_maps, core_ids=list(range(8)),
                                          trace=trace)
    # core c owns 128 t-columns per column-half (rank g of its quad):
    # half ha window = [seg0|seg1]; block = window[g*128:(g+1)*128]
    full = np.zeros((2, D_MODEL, L), np.float32)
    for c in range(8):
        b = 0 if c in (0, 1, 4, 5) else 1
        g = (c & 1) + 2 * (c >> 2)
        om = res.results[c]["out_m"]
        for ha in range(2):
            segs = _HSEGS[ha]
            c0 = segs[g // 2][0] + (g % 2) * 128
            full[b, :, c0:c0 + 128] = om[:, ha * 128:(ha + 1) * 128]
    out = np.ascontiguousarray(full.transpose(0, 2, 1))
    return out, res


def kernel(**inputs):
    out, _ = run(inputs, trace=False)
    return out


# revision 22
# speedup vs baseline: 1.1056x; 1.1056x over previous
"""BiMamba (bidirectional Mamba block + LN + FFN) Trainium2 Bass kernel.

Sharding (8 cores): 4 scan-sequences (fwd/bwd x batch, bwd fed host-flipped x)
x 2 halves of d_inner. Feature-on-partitions / time-on-free throughout.

v3 changes vs v2:
 - NO gpsimd elementwise ops: gpsimd shares an exclusive SBUF port pair with
   the DVE; any gpsimd tensor op fully blocks concurrent DVE 2-src ops
   (measured 3.2x slowdown). gpsimd only triggers collectives now.
 - scan-phase elementwise quad-batched: one [P, 4096] op covers the 4
   d-blocks of a half (b-mult / scan / prod-mult); independent streams are
   separated in the scan by forcing a=0 at each stream's first column.
   B/C row-broadcast tiles are read 4x via a zero-stride AP view.
 - y += D*xc folded into the scan PSUM accumulation with diag(D) matmuls
   (host passes Ddiag_t); yg merge is a single quad TT per half.
 - back-end collectives split in row-halves (AR) and the ReduceScatter in
   two pieces so collective latency overlaps compute; final output rows
   remapped accordingly on the host.
 - out_proj/FFN weight preloads: ow trickles during the scan, w1/w2 are
   DMA'd at the start of the back end (hidden under out_proj + AllReduce).
"""
import sys, os, types, contextlib, ctypes

sys.path.insert(0, "/opt/trn_rl_repo")
import numpy as np
import ml_dtypes

BF16 = ml_dtypes.bfloat16

D_MODEL = 1024
D_STATE = 16
D_CONV = 4
D_INNER = 2048
DT_RANK = 64
L = 1024
HALF = D_INNER // 2          # 1024 d_inner per core
P = 128
NJ = HALF // P               # 8 d-blocks per core half
TCH = 512                    # matmul t-chunk
NT = L // TCH
KD = D_MODEL // P            # 8 k-chunks over d_model
FF_SLICE = 1024              # ffn hidden slice per core
QL = 4 * L                   # quad free width (4 d-blocks)

_QUADS = [[0, 1, 4, 5], [2, 3, 6, 7]]
_PAIRS = [[0, 1], [2, 3], [4, 5], [6, 7]]

# column segments for the two reversal-symmetric back-end chunks
_HSEGS = [((0, 256), (768, 1024)), ((256, 512), (512, 768))]


def _install_ntff_hook_shim(so_path="/opt/axon/libaxon_pjrt.so"):
    if "antenv.axon_hooks" in sys.modules:
        return
    try:
        lib = ctypes.CDLL(so_path)
    except OSError:
        return
    if not hasattr(lib, "axon_start_nrt_profile"):
        return
    lib.axon_start_nrt_profile.argtypes = [ctypes.POINTER(ctypes.c_int64), ctypes.c_size_t]
    lib.axon_start_nrt_profile.restype = ctypes.c_int64
    lib.axon_stop_nrt_profile.argtypes = [ctypes.c_char_p]
    lib.axon_stop_nrt_profile.restype = ctypes.c_int64

    @contextlib.contextmanager
    def _hook(output_dir, device_ids):
        import jax
        jax.devices()
        if device_ids:
            ids = (ctypes.c_int64 * len(device_ids))(*device_ids)
            rc = lib.axon_start_nrt_profile(ids, len(device_ids))
        else:
            rc = lib.axon_start_nrt_profile(None, 0)
        if rc != 0:
            raise RuntimeError(f"axon_start_nrt_profile rc={rc}")
        try:
            yield
        finally:
            n = lib.axon_stop_nrt_profile(str(output_dir).encode())
            print(f"profile: {n} file(s) written to {output_dir}", file=sys.stderr)

    mod = types.ModuleType("antenv.axon_hooks")
    mod.get_axon_ntff_profile_hook = lambda: _hook
    mod.set_axon_ntff_profile_hook = lambda h: None
    sys.modules["antenv.axon_hooks"] = mod


def _build_nc():
    from concourse import bacc, tile, mybir

    f32 = mybir.dt.float32
    bf16 = mybir.dt.bfloat16
    u8 = mybir.dt.uint8
    Alu = mybir.AluOpType
    Act = mybir.ActivationFunctionType

    nc = bacc.Bacc("TRN2", target_bir_lowering=False, debug=False, num_devices=8)

    def din(name, shape, dt=None):
        return nc.dram_tensor(name, list(shape), dt or f32, kind="ExternalInput").ap()

    xT = din("xT", (D_MODEL, L), bf16)
    w_in_h = din("w_in_h", (2 * NJ, P, D_MODEL), bf16)   # [fb][kp, k*128+mp]
    convw_cols = din("convw_cols", (P, NJ * D_CONV))
    convb_cols = din("convb_cols", (P, NJ))
    xpw_t = din("xpw_t", (NJ, P, P), bf16)               # cols: dt64|B16|pad|C16|pad
    dtw_t = din("dtw_t", (NJ, DT_RANK, P), bf16)
    dtb_cols = din("dtb_cols", (P, NJ))
    A_cols = din("A_cols", (P, NJ * D_STATE))
    Ddiag_cols = din("Ddiag_cols", (P, NJ * P), bf16)    # diag(D) blocks side-by-side
    ow_h = din("ow_h", (NJ, P, D_MODEL), bf16)           # [k][kp, dm]
    lng_cols = din("lng_cols", (P, KD))
    lnb_cols = din("lnb_cols", (P, KD))
    w1_h = din("w1_h", (KD, P, 4 * D_MODEL), bf16)       # full [k][kp, h]
    b1_cols = din("b1_cols", (P, 4 * KD))
    w2_h = din("w2_h", (KD, P, 4 * D_MODEL), bf16)       # [g][kp, (q, dm)]
    b2_cols = din("b2_cols", (P, KD))
    ident_b = din("ident_b", (P, P), bf16)
    dirmask = din("dirmask", (P, TCH), u8)               # 1 on bwd cores
    onescol = din("onescol", (P, 1), bf16)               # 2^-10 (1/1024)
    onesrow = din("onesrow", (1, P), bf16)               # 1.0

    out_m = nc.dram_tensor("out_m", [D_MODEL, 2 * TCH // 4], f32,
                           kind="ExternalOutput").ap()

    with tile.TileContext(nc) as tc:
        with contextlib.ExitStack() as stk:
            cpool = stk.enter_context(tc.tile_pool(name="cpool", bufs=1))
            dram = stk.enter_context(tc.tile_pool(name="dram", bufs=1, space="DRAM"))

            def cload(src, shape, tag, dt=f32):
                t = cpool.tile(list(shape), dt, tag=tag, name=tag)
                nc.sync.dma_start(t[:], src)
                return t

            _consts = {}

            def load_consts():
                _consts["A_sb"] = cload(A_cols[:], (P, NJ * D_STATE), "A_sb")
                _consts["dtb_sb"] = cload(dtb_cols[:], (P, NJ), "dtb_sb")
                _consts["convb_sb"] = cload(convb_cols[:], (P, NJ), "convb_sb")
                _consts["convw_sb"] = cload(convw_cols[:], (P, NJ * D_CONV),
                                            "convw_sb")
                _consts["lng_sb"] = cload(lng_cols[:], (P, KD), "lng_sb")
                _consts["lnb_sb"] = cload(lnb_cols[:], (P, KD), "lnb_sb")
                _consts["b1_sb"] = cload(b1_cols[:], (P, 4 * KD), "b1_sb")
                _consts["b2_sb"] = cload(b2_cols[:], (P, KD), "b2_sb")
                _consts["ident_sb"] = cload(ident_b[:], (P, P), "ident_sb", bf16)
                _consts["dirmask_sb"] = cload(dirmask[:], (P, TCH),
                                              "dirmask_sb", u8)
                _consts["onescol_sb"] = cload(onescol[:], (P, 1), "onescol_sb",
                                              bf16)
                _consts["onesrow_sb"] = cload(onesrow[:], (1, P), "onesrow_sb",
                                              bf16)
                _consts["Ddiag_sb"] = cload(Ddiag_cols[:], (P, NJ * P),
                                            "Ddiag_sb", bf16)

            warm_in = dram.tile([1, 256], bf16, name="warm_in")
            warm_out = dram.tile([1, 256], bf16, name="warm_out")
            dbl_in_d = dram.tile([P, L], bf16, name="dbl_in_d")
            dbl_out_d = dram.tile([P, L], bf16, name="dbl_out_d")
            bcB_d = dram.tile([D_STATE, L], bf16, name="bcB_d")
            bcC_d = dram.tile([D_STATE, L], bf16, name="bcC_d")
            arm_in = [dram.tile([4, D_MODEL, TCH // 4], bf16, name=f"arm_in{h}")
                      for h in range(2)]
            rs_out = [dram.tile([D_MODEL, TCH // 4], bf16, name=f"rs_out{h}")
                      for h in range(2)]

            def mm_accum(ps, lw_list, rhs_of_k, n_k):
                for k in range(n_k):
                    nc.tensor.matmul(ps[:], lw_list[k][:], rhs_of_k(k),
                                     start=(k == 0), stop=(k == n_k - 1))

            # yg survives into the back end; sz/wv/delta live only front+scan
            per_pool = stk.enter_context(tc.tile_pool(name="per_pool", bufs=1))
            yg_q = [per_pool.tile([P, QL], bf16, tag=f"ygq{h}", name=f"ygq{h}")
                    for h in range(2)]

            # out_proj/FFN weight pool (tiles created lazily: ow during scan,
            # w1/w2 at back-end start)
            wq = stk.enter_context(tc.tile_pool(name="wq", bufs=1))

            def _wtiles(src, n_m, n_k, pfx, pool=None):
                pool = pool or wq
                rows, pend = [], []
                for m in range(n_m):
                    row = []
                    for k in range(n_k):
                        t = pool.tile([P, P], bf16, tag=f"{pfx}{m}_{k}",
                                      name=f"{pfx}{m}_{k}")
                        pend.append((t, src[k, m]))
                        row.append(t)
                    rows.append(row)
                return rows, pend

            # ---------------- P1..P4: produce xc, sz, delta, wv ----------------
            with tc.tile_pool(name="xc_pool", bufs=1) as xc_pool, \
                 tc.tile_pool(name="xt_pool", bufs=1) as xt_pool:
                xc_q = [xc_pool.tile([P, QL], bf16, tag=f"xcq{h}", name=f"xcq{h}")
                        for h in range(2)]
                sz_q = [xc_pool.tile([P, QL], bf16, tag=f"szq{h}", name=f"szq{h}")
                        for h in range(2)]
                wv_q = [xc_pool.tile([P, QL], bf16, tag=f"wvq{h}", name=f"wvq{h}")
                        for h in range(2)]
                delta_q = [xc_pool.tile([P, QL], bf16, tag=f"dq{h}",
                                        name=f"dq{h}") for h in range(2)]
                with tc.tile_pool(name="p1t", bufs=1) as p1t, \
                     tc.tile_pool(name="psumA", bufs=4, space="PSUM") as psumA:
                    # critical-path DMAs first: j0/j1 weights + x chunks, THEN
                    # the constant tables (not needed for ~35us)
                    _lw_pre = {}
                    for fb in range(2):
                        lw = p1t.tile([P, D_MODEL], bf16, tag="lwb",
                                      name=f"lwb{fb}", bufs=3)
                        nc.sync.dma_start(lw[:], w_in_h[fb])
                        _lw_pre[fb] = lw
                    xts = []
                    for k in range(KD):
                        xt_k = xt_pool.tile([P, L], bf16, tag=f"xt{k}",
                                            name=f"xt{k}")
                        nc.sync.dma_start(xt_k[:], xT[k * P:(k + 1) * P, :])
                        xts.append(xt_k)
                    load_consts()
                    A_sb = _consts["A_sb"]
                    dtb_sb = _consts["dtb_sb"]
                    convb_sb = _consts["convb_sb"]
                    convw_sb = _consts["convw_sb"]
                    lng_sb = _consts["lng_sb"]
                    lnb_sb = _consts["lnb_sb"]
                    b1_sb = _consts["b1_sb"]
                    b2_sb = _consts["b2_sb"]
                    ident_sb = _consts["ident_sb"]
                    dirmask_sb = _consts["dirmask_sb"]
                    onescol_sb = _consts["onescol_sb"]
                    onesrow_sb = _consts["onesrow_sb"]
                    Ddiag_sb = _consts["Ddiag_sb"]

                    def in_proj_block(fb):
                        if fb in _lw_pre:
                            lw = _lw_pre.pop(fb)
                        else:
                            lw = p1t.tile([P, D_MODEL], bf16, tag="lwb",
                                          name=f"lwb{fb}", bufs=3)
                            nc.sync.dma_start(lw[:], w_in_h[fb])
                        pss = []
                        for t in range(NT):
                            ps = psumA.tile([P, TCH], f32, tag="ps",
                                            name=f"inp{fb}_{t}")
                            for k in range(KD):
                                nc.tensor.matmul(
                                    ps[:], lw[:, k * P:(k + 1) * P],
                                    xts[k][:, t * TCH:(t + 1) * TCH],
                                    start=(k == 0), stop=(k == KD - 1))
                            pss.append(ps)
                        return pss

                    # P1: xc half + conv + silu -> xc_q slices
                    for j in range(NJ):
                        hb, s = j // 4, j % 4
                        xcp = p1t.tile([P, L + D_CONV - 1], bf16, tag="xcp",
                                       name=f"xcp{j}", bufs=2)
                        nc.vector.memset(xcp[:, 0:D_CONV - 1], 0.0)
                        for t, ps in enumerate(in_proj_block(j)):
                            nc.scalar.copy(
                                xcp[:, D_CONV - 1 + t * TCH:
                                    D_CONV - 1 + (t + 1) * TCH],
                                ps[:])
                        # aligned copy of the odd-shifted window: keeps every
                        # conv tap on a 4B-aligned stream (2x DVE mode)
                        xcp1 = p1t.tile([P, L + 2], bf16, tag="xcp1",
                                        name=f"xcp1_{j}", bufs=2)
                        nc.scalar.copy(xcp1[:], xcp[:, 1:L + 3])
                        cacc = p1t.tile([P, L], bf16, tag="cacc", name=f"cacc{j}",
                                        bufs=2)
                        nc.vector.tensor_scalar_mul(
                            cacc[:], xcp[:, 0:L],
                            convw_sb[:, j * D_CONV:j * D_CONV + 1])
                        for i, (tile, o) in enumerate(
                                ((xcp1, 0), (xcp, 2), (xcp1, 2)), start=1):
                            nc.vector.scalar_tensor_tensor(
                                cacc[:], tile[:, o:o + L],
                                convw_sb[:, j * D_CONV + i:j * D_CONV + i + 1],
                                cacc[:], Alu.mult, Alu.add)
                        nc.scalar.activation(xc_q[hb][:, s * L:(s + 1) * L],
                                             cacc[:], Act.Silu,
                                             bias=convb_sb[:, j:j + 1])

                    # P2: x_proj partial over own xc half -> pairwise AllReduce
                    dblp = p1t.tile([P, L], bf16, tag="dblp", name="dblp", bufs=1)
                    for t in range(NT):
                        ps = psumA.tile([P, TCH], f32, tag="ps", name=f"xproj{t}")
                        for k in range(NJ):
                            lw = p1t.tile([P, P], bf16, tag="xpw",
                                          name=f"xpw{t}_{k}", bufs=2)
                            nc.sync.dma_start(lw[:], xpw_t[k])
                            nc.tensor.matmul(
                                ps[:], lw[:],
                                xc_q[k // 4][:, (k % 4) * L + t * TCH:
                                             (k % 4) * L + (t + 1) * TCH],
                                start=(k == 0), stop=(k == NJ - 1))
                        nc.scalar.copy(dblp[:, t * TCH:(t + 1) * TCH], ps[:])
                    nc.sync.dma_start(dbl_in_d[:], dblp[:])
                    nc.gpsimd.collective_compute(
                        "AllReduce", Alu.add, replica_groups=_PAIRS,
                        ins=[dbl_in_d[:]], outs=[dbl_out_d[:]])

                    # P4: dt_proj + softplus -> delta_q; wv_q; B/C rows.
                    # Only j0/j1 (first scan sub-half) here; the rest is
                    # deferred into the wave loop (engines have slack there).
                    dbl_sb = xc_pool.tile([P, L], bf16, tag="dbl_sb",
                                          name="dbl_sb", bufs=1)
                    nc.sync.dma_start(dbl_sb[:], dbl_out_d[:])
                    bcB_sb = p1t.tile([D_STATE, L], bf16, tag="bcB_sb",
                                      name="bcB_sb", bufs=1)
                    nc.scalar.copy(bcB_sb[:], dbl_sb[DT_RANK:DT_RANK + D_STATE, :])
                    nc.sync.dma_start(bcB_d[:], bcB_sb[:])
                    bcC_sb = p1t.tile([D_STATE, L], bf16, tag="bcC_sb",
                                      name="bcC_sb", bufs=1)
                    nc.scalar.copy(bcC_sb[:], dbl_sb[96:96 + D_STATE, :])
                    nc.sync.dma_start(bcC_d[:], bcC_sb[:])
                    for j in range(2):
                        hb, s = j // 4, j % 4
                        lw = p1t.tile([DT_RANK, P], bf16, tag="dtw",
                                      name=f"dtw{j}", bufs=2)
                        nc.sync.dma_start(lw[:], dtw_t[j])
                        spt = p1t.tile([P, L], bf16, tag="spt",
                                       name=f"spt{j}", bufs=2)
                        for t in range(NT):
                            ps = psumA.tile([P, TCH], f32, tag="ps",
                                            name=f"dtp{j}_{t}")
                            nc.tensor.matmul(
                                ps[:], lw[:],
                                dbl_sb[0:DT_RANK, t * TCH:(t + 1) * TCH],
                                start=True, stop=True)
                            nc.scalar.activation(spt[:, t * TCH:(t + 1) * TCH],
                                                 ps[:], Act.Exp,
                                                 bias=dtb_sb[:, j:j + 1])
                        nc.scalar.activation(
                            delta_q[hb][:, s * L:(s + 1) * L],
                            spt[:], Act.Ln, bias=1.0)
                    nc.vector.tensor_tensor(wv_q[0][:, 0:2 * L],
                                            delta_q[0][:, 0:2 * L],
                                            xc_q[0][:, 0:2 * L], Alu.mult)

                ow_big = [wq.tile([P, D_MODEL], bf16, tag=f"owb{k}",
                                  name=f"owb{k}") for k in range(NJ)]
                ow_pending = [(ow_big[k], ow_h[k]) for k in range(NJ)]

                # ------------- P5: scan (pair-width waves, deferred z/dt) ----
                with tc.tile_pool(name="pscan", bufs=1, space="PSUM") as pscan, \
                     tc.tile_pool(name="zps", bufs=2, space="PSUM") as zps, \
                     tc.tile_pool(name="tpool", bufs=1) as tpool:

                    def dtsp_thunk(j):
                        def f():
                            hb, s = j // 4, j % 4
                            lw = tpool.tile([DT_RANK, P], bf16, tag="dtwl",
                                            name=f"dtwl{j}", bufs=2)
                            nc.sync.dma_start(lw[:], dtw_t[j])
                            spt = tpool.tile([P, L], bf16, tag="sptl",
                                             name=f"sptl{j}", bufs=2)
                            for t in range(NT):
                                ps = zps.tile([P, TCH], f32, tag="zp",
                                              name=f"dtpl{j}_{t}")
                                nc.tensor.matmul(
                                    ps[:], lw[:],
                                    dbl_sb[0:DT_RANK, t * TCH:(t + 1) * TCH],
                                    start=True, stop=True)
                                nc.scalar.activation(
                                    spt[:, t * TCH:(t + 1) * TCH], ps[:],
                                    Act.Exp, bias=dtb_sb[:, j:j + 1])
                            nc.scalar.activation(
                                delta_q[hb][:, s * L:(s + 1) * L], spt[:],
                                Act.Ln, bias=1.0)
                        return f

                    def wv_thunk(hb2):
                        def f():
                            hb, hf = hb2 // 2, hb2 % 2
                            sl = slice(hf * 2 * L, (hf + 1) * 2 * L)
                            nc.vector.tensor_tensor(wv_q[hb][:, sl],
                                                    delta_q[hb][:, sl],
                                                    xc_q[hb][:, sl], Alu.mult)
                        return f

                    def z_thunk(j):
                        def f():
                            hb, s = j // 4, j % 4
                            lw = tpool.tile([P, D_MODEL], bf16, tag="zlwb",
                                            name=f"zlwb{j}", bufs=2)
                            nc.sync.dma_start(lw[:], w_in_h[NJ + j])
                            for t in range(NT):
                                ps = zps.tile([P, TCH], f32, tag="zp",
                                              name=f"zin{j}_{t}")
                                for k in range(KD):
                                    nc.tensor.matmul(
                                        ps[:], lw[:, k * P:(k + 1) * P],
                                        xts[k][:, t * TCH:(t + 1) * TCH],
                                        start=(k == 0), stop=(k == KD - 1))
                                nc.scalar.activation(
                                    sz_q[hb][:, s * L + t * TCH:
                                              s * L + (t + 1) * TCH],
                                    ps[:], Act.Silu)
                        return f

                    thunks = [z_thunk(0), z_thunk(1), dtsp_thunk(2),
                              dtsp_thunk(3), wv_thunk(1), z_thunk(2),
                              z_thunk(3), dtsp_thunk(4), dtsp_thunk(5),
                              dtsp_thunk(6), dtsp_thunk(7), wv_thunk(2),
                              z_thunk(4), z_thunk(5), wv_thunk(3),
                              z_thunk(6), z_thunk(7)]

                    # warm up the quad ring so the first real RS isn't cold
                    nc.gpsimd.collective_compute(
                        "AllReduce", Alu.add, replica_groups=_QUADS,
                        ins=[warm_in[:]], outs=[warm_out[:]])
                    for hb2 in range(4):
                        hb, hf = hb2 // 2, hb2 % 2
                        off = hf * 2 * L
                        yps2 = pscan.tile([P, 2 * L], f32, tag="yps2",
                                          name=f"yps2_{hb2}", bufs=1)
                        for s2 in range(2):
                            j = hb * 4 + hf * 2 + s2
                            for c2 in range(2):
                                psl = slice((s2 * 2 + c2) * TCH,
                                            (s2 * 2 + c2 + 1) * TCH)
                                nc.tensor.matmul(
                                    yps2[:, psl], Ddiag_sb[:, j * P:(j + 1) * P],
                                    xc_q[hb][:, off + s2 * L + c2 * TCH:
                                              off + s2 * L + (c2 + 1) * TCH],
                                    start=True, stop=False)
                        for n in range(D_STATE):
                            Bbc = tpool.tile([P, L], bf16, tag="Bbc",
                                             name=f"Bbc{hb2}_{n}", bufs=2)
                            nc.sync.dma_start(
                                Bbc[:],
                                bcB_d[n:n + 1, :].partition_broadcast(P).squeeze(1))
                            Cbc = tpool.tile([P, L], bf16, tag="Cbc",
                                             name=f"Cbc{hb2}_{n}", bufs=2)
                            nc.sync.dma_start(
                                Cbc[:],
                                bcC_d[n:n + 1, :].partition_broadcast(P).squeeze(1))
                            a_2 = tpool.tile([P, 2 * L], bf16, tag="a_2",
                                             name=f"a{hb2}_{n}", bufs=2)
                            for s2 in range(2):
                                j = hb * 4 + hf * 2 + s2
                                nc.scalar.activation(
                                    a_2[:, s2 * L:(s2 + 1) * L],
                                    delta_q[hb][:, off + s2 * L:
                                                off + (s2 + 1) * L],
                                    Act.Exp,
                                    scale=A_sb[:, j * D_STATE + n:
                                               j * D_STATE + n + 1])
                            b_2 = tpool.tile([P, 2 * L], bf16, tag="b_2",
                                             name=f"b{hb2}_{n}", bufs=1)
                            nc.vector.tensor_tensor(
                                b_2[:], wv_q[hb][:, off:off + 2 * L],
                                Bbc[:].unsqueeze(1).broadcast_to([P, 2, L]),
                                Alu.mult)
                            h_2 = tpool.tile([P, 2 * L], bf16, tag="h_2",
                                             name=f"h{hb2}_{n}", bufs=2)
                            for s2 in range(2):
                                nc.vector.tensor_tensor_scan(
                                    h_2[:, s2 * L:(s2 + 1) * L],
                                    a_2[:, s2 * L:(s2 + 1) * L],
                                    b_2[:, s2 * L:(s2 + 1) * L], 0.0,
                                    Alu.mult, Alu.add)
                            prod = tpool.tile([P, 2 * L], bf16, tag="prod",
                                              name=f"p{hb2}_{n}", bufs=2)
                            nc.vector.tensor_tensor(
                                prod[:], h_2[:],
                                Cbc[:].unsqueeze(1).broadcast_to([P, 2, L]),
                                Alu.mult)
                            for c in range(4):
                                psl = slice(c * TCH, (c + 1) * TCH)
                                nc.tensor.matmul(yps2[:, psl], ident_sb[:],
                                                 prod[:, psl], start=False,
                                                 stop=(n == D_STATE - 1))
                            if n % 2 == 0 and thunks:
                                thunks.pop(0)()
                            for wt, srcap in ow_pending[:1]:
                                nc.sync.dma_start(wt[:], srcap)
                            del ow_pending[:1]
                        yb2 = tpool.tile([P, 2 * L], bf16, tag="yb2",
                                         name=f"yb{hb2}", bufs=1)
                        nc.scalar.copy(yb2[:], yps2[:])
                        nc.vector.tensor_tensor(
                            yg_q[hb][:, off:off + 2 * L], yb2[:],
                            sz_q[hb][:, off:off + 2 * L], Alu.mult)

            # ------- P6..P8: out_proj + column-RS, then local LN + full FFN ----
            # One quad ReduceScatter (cc_dim=Free) per column-half: each core
            # ends up owning 128 t-columns per half (256 total) with FULL dm,
            # then runs LN + the ENTIRE FFN locally. No AllReduce, no second
            # collective round-trip.
            with tc.tile_pool(name="p6t", bufs=1) as p6t, \
                 tc.tile_pool(name="wq2", bufs=1) as wq2, \
                 tc.tile_pool(name="psumB", bufs=2, space="PSUM") as psumB, \
                 tc.tile_pool(name="pstat", bufs=1, space="PSUM") as pstat:

                w1_big = [wq2.tile([P, 4 * D_MODEL], bf16, tag=f"w1b{k}",
                                   name=f"w1b{k}") for k in range(KD)]
                w2_big = [wq2.tile([P, 4 * D_MODEL], bf16, tag=f"w2b{g}",
                                   name=f"w2b{g}") for g in range(KD)]

                def out_proj_half(ha):
                    sel = slice(0, 4, 3) if ha == 0 else slice(1, 3, 1)
                    for m in range(NJ):
                        ps = psumB.tile([P, TCH], f32, tag="ps",
                                        name=f"op{ha}_{m}")
                        for k in range(NJ):
                            rhs = yg_q[k // 4][:, (k % 4) * L:
                                               (k % 4) * L + L].rearrange(
                                "p (a b) -> p a b", a=4)[:, sel, :]
                            nc.tensor.matmul(ps[:],
                                             ow_big[k][:, m * P:(m + 1) * P],
                                             rhs, start=(k == 0),
                                             stop=(k == NJ - 1))
                        msb = p6t.tile([P, TCH], bf16, tag="msb",
                                       name=f"msb{ha}_{m}", bufs=2)
                        nc.scalar.copy(msb[:], ps[:])
                        nc.vector.copy_predicated(msb[:], dirmask_sb[:],
                                                  ps[:, ::-1])
                        for q in range(4):
                            nc.sync.dma_start(
                                arm_in[ha][q, m * P:(m + 1) * P, :],
                                msb[:, q * (TCH // 4):(q + 1) * (TCH // 4)])
                    nc.gpsimd.collective_compute(
                        "ReduceScatter", Alu.add, replica_groups=_QUADS,
                        ins=[arm_in[ha][:]], outs=[rs_out[ha][:]])

                TO = 2 * TCH // 4            # 256 owned t-columns per core

                def ln_ffn():
                    mos = []
                    mu_ps = pstat.tile([1, TO], f32, tag="mu_ps", name="mu",
                                       bufs=1)
                    e2_ps = pstat.tile([1, TO], f32, tag="e2_ps", name="e2",
                                       bufs=1)
                    for m in range(KD):
                        mo = p6t.tile([P, TO], bf16, tag=f"mo{m}",
                                      name=f"mo{m}", bufs=1)
                        mos.append(mo)
                    HT = TO // 2
                    for ha in range(2):          # ha0 stats overlap ha1's RS
                        for m in range(KD):
                            mo = mos[m]
                            nc.sync.dma_start(
                                mo[:, ha * HT:(ha + 1) * HT],
                                rs_out[ha][m * P:(m + 1) * P, :])
                            sq = p6t.tile([P, HT], bf16, tag="sq",
                                          name=f"sq{ha}_{m}", bufs=2)
                            nc.scalar.activation(
                                sq[:], mo[:, ha * HT:(ha + 1) * HT],
                                Act.Square)
                            nc.tensor.matmul(
                                mu_ps[:, ha * HT:(ha + 1) * HT],
                                onescol_sb[:], mo[:, ha * HT:(ha + 1) * HT],
                                start=(m == 0), stop=(m == KD - 1))
                            nc.tensor.matmul(
                                e2_ps[:, ha * HT:(ha + 1) * HT],
                                onescol_sb[:], sq[:],
                                start=(m == 0), stop=(m == KD - 1))
                    m2 = p6t.tile([1, TO], f32, tag="m2", name="m2", bufs=1)
                    nc.scalar.activation(m2[:], mu_ps[:], Act.Square)
                    var_t = p6t.tile([1, TO], f32, tag="var_t", name="var",
                                     bufs=1)
                    nc.vector.tensor_tensor(var_t[:], e2_ps[:], m2[:],
                                            Alu.subtract)
                    eps_sb = p6t.tile([1, 1], f32, tag="eps_sb", name="eps",
                                      bufs=1)
                    nc.vector.memset(eps_sb[:], 1e-5)
                    std_t = p6t.tile([1, TO], f32, tag="std_t", name="std",
                                     bufs=1)
                    nc.scalar.activation(std_t[:], var_t[:], Act.Sqrt,
                                         bias=eps_sb[:])
                    rstd_b = p6t.tile([1, TO], bf16, tag="rstd_b", name="rstd",
                                      bufs=1)
                    with nc.allow_low_precision(reason="bf16 rstd broadcast"):
                        nc.vector.reciprocal(rstd_b[:], std_t[:])
                    mean_b = p6t.tile([1, TO], bf16, tag="mean_b", name="mean",
                                      bufs=1)
                    nc.scalar.copy(mean_b[:], mu_ps[:])
                    mean_ps = pstat.tile([P, TO], f32, tag="mean_ps",
                                         name="meanbc", bufs=1)
                    nc.tensor.matmul(mean_ps[:], onesrow_sb[:], mean_b[:],
                                     start=True, stop=True)
                    rstd_ps = pstat.tile([P, TO], f32, tag="rstd_ps",
                                         name="rstdbc", bufs=1)
                    nc.tensor.matmul(rstd_ps[:], onesrow_sb[:], rstd_b[:],
                                     start=True, stop=True)
                    mean_bc = p6t.tile([P, TO], bf16, tag="mean_bc",
                                       name="meanbcs", bufs=1)
                    nc.scalar.copy(mean_bc[:], mean_ps[:])
                    rstd_bc = p6t.tile([P, TO], bf16, tag="rstd_bc",
                                       name="rstdbcs", bufs=1)
                    nc.scalar.copy(rstd_bc[:], rstd_ps[:])

                    xns = []
                    for m in range(KD):
                        t1 = p6t.tile([P, TO], bf16, tag="lnt", name=f"lnt{m}",
                                      bufs=2)
                        nc.vector.tensor_tensor(t1[:], mos[m][:], mean_bc[:],
                                                Alu.subtract)
                        nc.vector.tensor_tensor(t1[:], t1[:], rstd_bc[:],
                                                Alu.mult)
                        xn = p6t.tile([P, TO], bf16, tag=f"xn{m}",
                                      name=f"xn{m}", bufs=1)
                        nc.vector.tensor_scalar(xn[:], t1[:], lng_sb[:, m:m + 1],
                                                lnb_sb[:, m:m + 1], Alu.mult,
                                                Alu.add)
                        xns.append(xn)

                    ffhs = []
                    for m in range(4 * KD):
                        ps = psumB.tile([P, TO], f32, tag="psf", name=f"f1_{m}")
                        for k in range(KD):
                            nc.tensor.matmul(ps[:],
                                             w1_big[k][:, m * P:(m + 1) * P],
                                             xns[k][:], start=(k == 0),
                                             stop=(k == KD - 1))
                        ffh = p6t.tile([P, TO], bf16, tag=f"ffh{m}",
                                       name=f"ffh{m}", bufs=1)
                        nc.scalar.activation(ffh[:], ps[:], Act.Gelu,
                                             bias=b1_sb[:, m:m + 1])
                        ffhs.append(ffh)
                    for m in range(KD):
                        ps = psumB.tile([P, TO], f32, tag="psf", name=f"f2_{m}")
                        for k2 in range(4 * KD):
                            nc.tensor.matmul(
                                ps[:],
                                w2_big[k2 // 4][:, (k2 % 4) * D_MODEL +
                                                m * P:(k2 % 4) * D_MODEL +
                                                (m + 1) * P],
                                ffhs[k2][:], start=(k2 == 0),
                                stop=(k2 == 4 * KD - 1))
                        fob = p6t.tile([P, TO], f32, tag="fob", name=f"fob{m}",
                                       bufs=2)
                        nc.vector.tensor_scalar_add(fob[:], ps[:],
                                                    b2_sb[:, m:m + 1])
                        nc.sync.dma_start(out_m[m * P:(m + 1) * P, :], fob[:])

                out_proj_half(0)
                # weights stream during out_proj(1) + the ReduceScatters
                for k in range(KD):
                    nc.sync.dma_start(w1_big[k][:], w1_h[k])
                out_proj_half(1)
                for g in range(KD):
                    nc.sync.dma_start(w2_big[g][:], w2_h[g])
                ln_ffn()

    nc.compile()
    return nc


def _prep_inputs(inputs):
    """Per-core input dicts. Core c: sequence s=c//2 (s>=2 => time-flipped x),
    d_inner half = c%2. The own half of d_inner is permuted FIRST in every
    d_inner-ordered tensor, so the device kernel is identical on all cores."""
    x = np.asarray(inputs["x"], dtype=np.float32)
    in_proj_w = np.asarray(inputs["in_proj_w"], dtype=np.float32)
    conv_w = np.asarray(inputs["conv_w"], dtype=np.float32)
    conv_b = np.asarray(inputs["conv_b"], dtype=np.float32)
    x_proj_w = np.asarray(inputs["x_proj_w"], dtype=np.float32)
    dt_proj_w = np.asarray(inputs["dt_proj_w"], dtype=np.float32)
    dt_proj_b = np.asarray(inputs["dt_proj_b"], dtype=np.float32)
    A = -np.exp(np.asarray(inputs["A_log"], dtype=np.float32))
    Dp = np.asarray(inputs["D"], dtype=np.float32)
    out_proj_w = np.asarray(inputs["out_proj_w"], dtype=np.float32)
    ln_g = np.asarray(inputs["ln_g"], dtype=np.float32)
    ln_b = np.asarray(inputs["ln_b"], dtype=np.float32)
    ff_w1 = np.asarray(inputs["ff_w1"], dtype=np.float32)
    ff_b1 = np.asarray(inputs["ff_b1"], dtype=np.float32)
    ff_w2 = np.asarray(inputs["ff_w2"], dtype=np.float32)
    ff_b2 = np.asarray(inputs["ff_b2"], dtype=np.float32)

    def cols(v):  # (N,) -> (P, N//P) per-partition column layout
        return np.ascontiguousarray(v.reshape(-1, P).T)

    def tile_w(w, KP, MP):  # (K, M) -> (K//KP, M//MP, KP, MP) bf16
        K, M = w.shape
        return np.ascontiguousarray(
            w.reshape(K // KP, KP, M // MP, MP).transpose(0, 2, 1, 3)
        ).astype(BF16)

    in_maps = []
    for c in range(8):
        s, half = c // 2, c % 2
        xb = x[s] if s < 2 else x[s - 2][::-1]
        perm = np.arange(D_INNER).reshape(2, HALF)
        own = np.concatenate([perm[half], perm[1 - half]])[:HALF]

        wxc = in_proj_w[:, own]                               # (1024, 1024)
        wz = in_proj_w[:, D_INNER + own]                      # (1024, 1024)
        w_in = np.concatenate([wxc, wz], axis=1)              # (1024, 2048)
        w_in_h = np.ascontiguousarray(
            w_in.reshape(KD, P, 2 * NJ, P).transpose(2, 1, 0, 3)
            .reshape(2 * NJ, P, D_MODEL)).astype(BF16)        # (16 fb, P, 1024)

        cw = conv_w[own]  # (1024, 4) -> (P, 8*4): col j*4+i = w[jP+p, i]
        convw_cols = np.ascontiguousarray(
            cw.reshape(NJ, P, D_CONV).transpose(1, 0, 2).reshape(P, NJ * D_CONV))

        g = (c & 1) + 2 * (c >> 2)

        Ddiag = np.hstack([np.diag(Dp[own][j * P:(j + 1) * P])
                           for j in range(NJ)]).astype(BF16)

        in_maps.append({
            "xT": np.ascontiguousarray(xb.T).astype(BF16),
            "w_in_h": w_in_h,
            "convw_cols": convw_cols,
            "convb_cols": cols(conv_b[own]),
            "xpw_t": np.ascontiguousarray(
                np.concatenate([
                    x_proj_w[own][:, :DT_RANK + D_STATE],
                    np.zeros((HALF, D_STATE), np.float32),
                    x_proj_w[own][:, DT_RANK + D_STATE:],
                    np.zeros((HALF, D_STATE), np.float32),
                ], axis=1).reshape(NJ, P, P)).astype(BF16),
            "dtw_t": np.ascontiguousarray(
                dt_proj_w[:, own].reshape(DT_RANK, NJ, P).transpose(1, 0, 2)
            ).astype(BF16),
            "dtb_cols": cols(dt_proj_b[own]),
            "A_cols": np.ascontiguousarray(
                A[own].reshape(NJ, P, D_STATE).transpose(1, 0, 2).reshape(
                    P, NJ * D_STATE)),
            "Ddiag_cols": Ddiag,
            "ow_h": np.ascontiguousarray(
                out_proj_w[own].reshape(NJ, P, D_MODEL)).astype(BF16),
            "lng_cols": cols(ln_g),
            "lnb_cols": cols(ln_b),
            "w1_h": np.ascontiguousarray(
                ff_w1.reshape(KD, P, 4 * D_MODEL)).astype(BF16),
            "b1_cols": cols(ff_b1),
            "w2_h": np.ascontiguousarray(
                ff_w2.reshape(KD, 4, P, D_MODEL).transpose(0, 2, 1, 3)
                .reshape(KD, P, 4 * D_MODEL)).astype(BF16),
            "b2_cols": cols(ff_b2),
            "ident_b": np.eye(P, dtype=np.float32).astype(BF16),
            "dirmask": np.full((P, TCH), 1 if s >= 2 else 0, np.uint8),
            "onescol": np.full((P, 1), 1.0 / 1024.0, np.float32).astype(BF16),
            "onesrow": np.ones((1, P), np.float32).astype(BF16),
        })
    return in_maps


_NC_CACHE = {}


def _get_nc():
    if "nc" not in _NC_CACHE:
        _NC_CACHE["nc"] = _build_nc()
    return _NC_CACHE["nc"]


def run(inputs, trace=False):
    _install_ntff_hook_shim()
    from concourse import bass_utils
    nc = _get_nc()
    in_maps = _prep_inputs(inputs)
    res = bass_utils.run_bass_kernel_spmd(nc, in_maps, core_ids=list(range(8)),
                                          trace=trace)
    # core c owns 128 t-columns per column-half (rank g of its quad):
    # half ha window = [seg0|seg1]; block = window[g*128:(g+1)*128]
    full = np.zeros((2, D_MODEL, L), np.float32)
    for c in range(8):
        b = 0 if c in (0, 1, 4, 5) else 1
        g = (c & 1) + 2 * (c >> 2)
        om = res.results[c]["out_m"]
        for ha in range(2):
            segs = _HSEGS[ha]
            c0 = segs[g // 2][0] + (g % 2) * 128
            full[b, :, c0:c0 + 128] = om[:, ha * 128:(ha + 1) * 128]
    out = np.ascontiguousarray(full.transpose(0, 2, 1))
    return out, res


def kernel(**inputs):
    out, _ = run(inputs, trace=False)
    return out
